# revision 1
# baseline (speedup 1.0000x reference)
# Trainium2 Bass kernel for nn_LocalLayer (banded/local linear layer).
#
#   reference: y = x @ W.T + b
#     x [8192, 4096] f32, W [4096, 4096] f32 (block-banded: 256 windows x 16
#     outputs, window k reads inputs [16k-32, 16k+32) clipped to [0, 4096)),
#     b [4096] f32.
#
# Strategy (8 NeuronCores, data-parallel over batch):
#   - Host: transpose x -> xt [4096, 8192], shard batch 8 ways, zero-pad rows
#     by 32 (top) / 96 (bottom) -> per-core xt_pad [4224, 1024].  The -32 row
#     shift makes every output tile's 176-wide input window split into exactly
#     two aligned 128-row chunks.
#   - Host: gather W's band into compact stationary blocks:
#       wc1[:, O*128+j][i] = W[128O+j, 128O-32+i]   (i in 0..127)
#       wc2[:, O*128+j][i] = W[128O+j, 128O+96+i]   (zero outside band/range)
#   - Precision: pseudo-fp32 via bf16 hi/lo split (x = xh + xl, w = wh + wl);
#     y ~= xh wh + xh wl + xl wh accumulated in fp32 PSUM (drop xl wl,
#     ~2^-16 relative).  bf16 matmuls stream 1 row/cycle with fast weight
#     load; fp32/f32r matmuls are 3-5x slower on the PE.
#   - Device (per core): for each output tile O (32) and batch chunk bc (2):
#       psum[128, 512] f32 = 6 accumulating bf16 matmuls (2 K-chunks x 3 terms)
#       sbuf = psum + bias[:, O]   (ScalarE / VectorE alternating)
#       DMA out -> yt [4096, 1024] f32
#   - Host: y = concat([yt_c.T for c in cores]).
#
# kernel() is self-contained: shapes/sharding hardcoded, no file reads.

import ml_dtypes
import numpy as np

import concourse.mybir as mybir
import concourse.tile as tile
from concourse import bacc
from concourse.bass_utils import run_bass_kernel_spmd

BF16 = ml_dtypes.bfloat16

BATCH = 8192
IN = 4096
N_CORES = 8
B_CORE = BATCH // N_CORES          # 1024
O_TILES = IN // 128                # 32
PAD_TOP = 32
ROWS_PAD = (O_TILES + 1) * 128     # 4224
BC = 512                           # batch chunk (one PSUM bank of f32)
N_BC = B_CORE // BC                # 2

_NC_CACHE = {}


def _build_nc(xt_bufs=12, out_bufs=6, psum_bufs=8):
    key = (xt_bufs, out_bufs, psum_bufs)
    if key in _NC_CACHE:
        return _NC_CACHE[key]
    f32 = mybir.dt.float32
    bf16 = mybir.dt.bfloat16
    nc = bacc.Bacc("TRN2", target_bir_lowering=False, debug=False)
    xh_d = nc.dram_tensor("xh", [ROWS_PAD, B_CORE], bf16, kind="ExternalInput")
    xl_d = nc.dram_tensor("xl", [ROWS_PAD, B_CORE], bf16, kind="ExternalInput")
    w_names = ["w1h", "w1l", "w2h", "w2l"]
    w_d = {n: nc.dram_tensor(n, [128, IN], bf16, kind="ExternalInput") for n in w_names}
    bias_d = nc.dram_tensor("bias", [128, O_TILES], f32, kind="ExternalInput")
    yt_d = nc.dram_tensor("yt", [IN, B_CORE], f32, kind="ExternalOutput")

    with tile.TileContext(nc) as tc:
        with (
            tc.tile_pool(name="consts", bufs=1) as cpool,
            tc.tile_pool(name="xt", bufs=xt_bufs) as xpool,
            tc.tile_pool(name="psum", bufs=psum_bufs, space="PSUM") as ppool,
            tc.tile_pool(name="out", bufs=out_bufs) as opool,
        ):
            w_t = {}
            for n in w_names:
                w_t[n] = cpool.tile([128, IN], bf16, name=n, tag=n)
                # chunked so the first matmuls only wait on the first columns
                for q in range(4):
                    qs = slice(q * (IN // 4), (q + 1) * (IN // 4))
                    nc.sync.dma_start(w_t[n][:, qs], w_d[n].ap()[:, qs])
            bias_t = cpool.tile([128, O_TILES], f32)
            nc.sync.dma_start(bias_t, bias_d.ap())

            hts, lts = {}, {}
            for O in range(O_TILES + 1):
                th = xpool.tile([128, B_CORE], bf16, tag="xh")
                tl = xpool.tile([128, B_CORE], bf16, tag="xl")
                nc.sync.dma_start(th, xh_d.ap()[O * 128:(O + 1) * 128, :])
                nc.sync.dma_start(tl, xl_d.ap()[O * 128:(O + 1) * 128, :])
                hts[O], lts[O] = th, tl
                if O == 0:
                    continue
                Oc = O - 1
                osl = slice(Oc * 128, (Oc + 1) * 128)
                out_t = opool.tile([128, B_CORE], f32, tag="out")
                # one stationary load per term, streamed over both batch
                # chunks back-to-back so matmuls pipeline at ~N cycles
                terms = [
                    ("w1h", hts[Oc]), ("w1h", lts[Oc]), ("w1l", hts[Oc]),
                    ("w2h", hts[Oc + 1]), ("w2h", lts[Oc + 1]), ("w2l", hts[Oc + 1]),
                ]
                pss = [
                    ppool.tile([128, BC], f32, tag="ps", name=f"ps_{Oc}_{i}")
                    for i in range(N_BC)
                ]
                for ti, (wn, xt_t) in enumerate(terms):
                    for bc in range(N_BC):
                        nc.tensor.matmul(
                            pss[bc],
                            w_t[wn][:, osl],
                            xt_t[:, bc * BC:(bc + 1) * BC],
                            start=(ti == 0),
                            stop=(ti == len(terms) - 1),
                        )
                for bc in range(N_BC):
                    bsl = slice(bc * BC, (bc + 1) * BC)
                    nc.scalar.add(out_t[:, bsl], pss[bc], bias_t[:, Oc:Oc + 1])
                nc.sync.dma_start(yt_d.ap()[osl, :], out_t)

    nc.compile()
    _NC_CACHE[key] = nc
    return nc


def _band_gather(W, shift):
    """wc[i, O*128+j] = W[128O+j, 128O+shift+i], zero outside [0, IN)."""
    i = np.arange(128)[:, None, None]
    O = np.arange(O_TILES)[None, :, None]
    j = np.arange(128)[None, None, :]
    o_idx = np.broadcast_to(128 * O + j, (128, O_TILES, 128))
    f = 128 * O + shift + i
    wc = np.where(
        (f >= 0) & (f < IN), W[o_idx, np.clip(f, 0, IN - 1)], np.float32(0)
    )
    return wc.reshape(128, O_TILES * 128)


def _split_bf16(a):
    hi = a.astype(BF16)
    lo = (a - hi.astype(np.float32)).astype(BF16)
    return hi, lo


def kernel(x, W, b, mask=None):
    x = np.asarray(x, dtype=np.float32)
    W = np.asarray(W, dtype=np.float32)

    wc1 = _band_gather(W, -PAD_TOP)
    wc2 = _band_gather(W, 128 - PAD_TOP)
    w1h, w1l = _split_bf16(wc1)
    w2h, w2l = _split_bf16(wc2)
    bias = np.ascontiguousarray(
        np.asarray(b, dtype=np.float32).reshape(O_TILES, 128).T
    )

    xt = x.T  # [4096, 8192] view
    in_maps = []
    for c in range(N_CORES):
        sh = np.zeros((ROWS_PAD, B_CORE), np.float32)
        sh[PAD_TOP:PAD_TOP + IN, :] = xt[:, c * B_CORE:(c + 1) * B_CORE]
        xh, xl = _split_bf16(sh)
        in_maps.append(
            {"xh": xh, "xl": xl, "w1h": w1h, "w1l": w1l, "w2h": w2h,
             "w2l": w2l, "bias": bias}
        )

    nc = _build_nc()
    res = run_bass_kernel_spmd(nc, in_maps, core_ids=list(range(N_CORES)))
    y = np.concatenate(
        [np.asarray(r["yt"]).T for r in res.results], axis=0
    )
    return np.ascontiguousarray(y)


if __name__ == "__main__":
    rng = np.random.default_rng(0)
    x = rng.standard_normal((BATCH, IN), dtype=np.float32)
    W = rng.standard_normal((IN, IN), dtype=np.float32)
    b = rng.standard_normal(IN, dtype=np.float32)
    y = kernel(x, W, b)
    print(y.shape, y.dtype)



# revision 2
# speedup vs baseline: 2.1769x; 2.1769x over previous
# Trainium2 Bass kernel for nn_LocalLayer (banded/local linear layer).
#
#   reference: y = x @ W.T + b
#     x [8192, 4096] f32, W [4096, 4096] f32 (block-banded: 256 windows x 16
#     outputs, window k reads inputs [16k-32, 16k+32) clipped to [0, 4096)),
#     b [4096] f32.
#
# Strategy (8 NeuronCores, data-parallel over batch):
#   - Precision: single bf16 matmul term (x_bf16 @ W_bf16) accumulated in
#     f32 PSUM, bias added during PSUM evacuation, output stored bf16 and
#     converted to f32 on host.  Measured rel err ~4e-3 vs the 2e-2 gate.
#   - Host: per core, transpose + zero-pad the batch shard by 32 rows (top)
#     to xt_pad [33*128, 1024] bf16, then pack row-block-major into
#     xh [128, 33*1024] so big column-range DMAs are per-partition
#     contiguous.  Band of W gathered into two stationary chunk tensors
#     (w1[:, O*128+j][i] = W[128O+j, 128O-32+i], w2 same at +96).
#   - Device (per core): x arrives in 7 large DMAs (0.5-1.5 MB) on the Sync
#     HWDGE ring; outputs leave in 8 x 1MB DMAs on the Scalar HWDGE ring;
#     weights via GpSimd SWDGE.  Per output tile O (32): 4 bf16 matmuls
#     (2 K-chunks x 2 batch halves) into 2 PSUM banks; evacuation split
#     between ScalarE (batch half 0) and VectorE (half 1), each doing
#     psum + bias -> bf16 out tile.
#   - Host: y = concat of per-core unpacked [1024, 4096] tiles, cast f32.
#
# kernel() is self-contained: shapes/sharding hardcoded, no file reads.

import ml_dtypes
import numpy as np

import concourse.mybir as mybir
import concourse.tile as tile
from concourse import bacc
from concourse.bass_utils import run_bass_kernel_spmd

BF16 = ml_dtypes.bfloat16

BATCH = 8192
IN = 4096
N_CORES = 8
B_CORE = BATCH // N_CORES          # 1024
O_TILES = IN // 128                # 32
PAD_TOP = 32
NBLK = O_TILES + 1                 # 33 x row-blocks of 128
BC = 512                           # batch chunk (one PSUM bank of f32)
N_BC = B_CORE // BC                # 2
OG = 4                             # output tiles per out-DMA group
# x DMA piece sizes in row-blocks (sum = NBLK); small first so compute starts
X_PIECES = [2, 3, 4, 6, 6, 6, 6]

_NC_CACHE = {}


def _build_nc():
    if "nc" in _NC_CACHE:
        return _NC_CACHE["nc"]
    f32 = mybir.dt.float32
    bf16 = mybir.dt.bfloat16
    nc = bacc.Bacc("TRN2", target_bir_lowering=False, debug=False)
    xh_d = nc.dram_tensor("xh", [128, NBLK * B_CORE], bf16, kind="ExternalInput")
    w1_d = nc.dram_tensor("w1", [128, IN], bf16, kind="ExternalInput")
    w2_d = nc.dram_tensor("w2", [128, IN], bf16, kind="ExternalInput")
    bias_d = nc.dram_tensor("bias", [128, O_TILES], f32, kind="ExternalInput")
    yt_d = nc.dram_tensor("yt", [128, O_TILES * B_CORE], bf16,
                          kind="ExternalOutput")

    with tile.TileContext(nc) as tc:
        with (
            tc.tile_pool(name="consts", bufs=1) as cpool,
            tc.tile_pool(name="psum", bufs=8, space="PSUM") as ppool,
            tc.tile_pool(name="out", bufs=3) as opool,
        ):
            # weights + bias via SWDGE (separate queue path from the x stream)
            w1_t = cpool.tile([128, IN], bf16, name="w1", tag="w1")
            w2_t = cpool.tile([128, IN], bf16, name="w2", tag="w2")
            nc.gpsimd.dma_start(w1_t, w1_d.ap())
            nc.gpsimd.dma_start(w2_t, w2_d.ap())
            bias_t = cpool.tile([128, O_TILES], f32, name="bias", tag="bias")
            nc.gpsimd.dma_start(bias_t, bias_d.ap())

            # x pieces: large column-range DMAs on the Sync HWDGE ring
            xp_t = []          # per piece tile
            blk_loc = {}       # row-block -> (piece idx, local col offset)
            c0 = 0
            for pi, nb in enumerate(X_PIECES):
                t = cpool.tile([128, nb * B_CORE], bf16, name=f"xp{pi}",
                               tag=f"xp{pi}")
                nc.sync.dma_start(
                    t, xh_d.ap()[:, c0 * B_CORE:(c0 + nb) * B_CORE]
                )
                xp_t.append(t)
                for j in range(nb):
                    blk_loc[c0 + j] = (pi, j * B_CORE)
                c0 += nb

            def xblk(b, bc):
                pi, off = blk_loc[b]
                return xp_t[pi][:, off + bc * BC: off + (bc + 1) * BC]

            for g in range(O_TILES // OG):
                og_t = opool.tile([128, OG * B_CORE], bf16, tag="out")
                for lo in range(OG):
                    O = g * OG + lo
                    osl = slice(O * 128, (O + 1) * 128)
                    pss = [
                        ppool.tile([128, BC], f32, tag="ps", name=f"ps_{O}_{i}")
                        for i in range(N_BC)
                    ]
                    # one stationary load per chunk, both batch halves
                    for bc in range(N_BC):
                        nc.tensor.matmul(pss[bc], w1_t[:, osl], xblk(O, bc),
                                         start=True, stop=False)
                    for bc in range(N_BC):
                        nc.tensor.matmul(pss[bc], w2_t[:, osl], xblk(O + 1, bc),
                                         start=False, stop=True)
                    # evacuation: psum + bias -> bf16, split ACT / DVE
                    ob = lo * B_CORE
                    nc.scalar.add(og_t[:, ob:ob + BC], pss[0],
                                  bias_t[:, O:O + 1])
                    nc.vector.tensor_scalar_add(og_t[:, ob + BC:ob + 2 * BC],
                                                pss[1], bias_t[:, O:O + 1])
                # 1MB output DMA on the Scalar HWDGE ring
                nc.scalar.dma_start(
                    yt_d.ap()[:, g * OG * B_CORE:(g + 1) * OG * B_CORE], og_t
                )

    nc.compile()
    _NC_CACHE["nc"] = nc
    return nc


def _band_gather(W, shift):
    """wc[i, O*128+j] = W[128O+j, 128O+shift+i], zero outside [0, IN)."""
    i = np.arange(128)[:, None, None]
    O = np.arange(O_TILES)[None, :, None]
    j = np.arange(128)[None, None, :]
    o_idx = np.broadcast_to(128 * O + j, (128, O_TILES, 128))
    f = 128 * O + shift + i
    wc = np.where(
        (f >= 0) & (f < IN), W[o_idx, np.clip(f, 0, IN - 1)], np.float32(0)
    )
    return wc.reshape(128, O_TILES * 128)


def kernel(x, W, b, mask=None):
    x = np.asarray(x, dtype=np.float32)
    W = np.asarray(W, dtype=np.float32)

    w1 = _band_gather(W, -PAD_TOP).astype(BF16)
    w2 = _band_gather(W, 128 - PAD_TOP).astype(BF16)
    bias = np.ascontiguousarray(
        np.asarray(b, dtype=np.float32).reshape(O_TILES, 128).T
    )

    xt = x.T  # [4096, 8192] view
    in_maps = []
    for c in range(N_CORES):
        sh = np.zeros((NBLK * 128, B_CORE), np.float32)
        sh[PAD_TOP:PAD_TOP + IN, :] = xt[:, c * B_CORE:(c + 1) * B_CORE]
        xh = np.ascontiguousarray(
            sh.astype(BF16).reshape(NBLK, 128, B_CORE).transpose(1, 0, 2)
        ).reshape(128, NBLK * B_CORE)
        in_maps.append({"xh": xh, "w1": w1, "w2": w2, "bias": bias})

    nc = _build_nc()
    res = run_bass_kernel_spmd(nc, in_maps, core_ids=list(range(N_CORES)))
    outs = []
    for r in res.results:
        yt = np.asarray(r["yt"]).reshape(128, O_TILES, B_CORE)
        outs.append(
            yt.transpose(2, 1, 0).reshape(B_CORE, IN).astype(np.float32)
        )
    return np.ascontiguousarray(np.concatenate(outs, axis=0))


if __name__ == "__main__":
    rng = np.random.default_rng(0)
    x = rng.standard_normal((BATCH, IN), dtype=np.float32)
    W = rng.standard_normal((IN, IN), dtype=np.float32)
    b = rng.standard_normal(IN, dtype=np.float32)
    y = kernel(x, W, b)
    print(y.shape, y.dtype)


# revision 4
# speedup vs baseline: 2.3704x; 1.0889x over previous
# Trainium2 Bass kernel for nn_LocalLayer (banded/local linear layer).
#
#   reference: y = x @ W.T + b
#     x [8192, 4096] f32, W [4096, 4096] f32 (block-banded: 256 windows x 16
#     outputs, window k reads inputs [16k-32, 16k+32) clipped to [0, 4096)),
#     b [4096] f32.
#
# Strategy (8 NeuronCores, data-parallel over batch):
#   - Precision: single bf16 matmul term (x_bf16 @ W_bf16) accumulated in
#     f32 PSUM, bias added during PSUM evacuation, output stored bf16 and
#     converted to f32 on host.  Measured rel err ~4e-3 vs the 2e-2 gate.
#   - Host: per core, transpose + zero-pad the batch shard by 32 rows (top)
#     to xt_pad [33*128, 1024] bf16, then pack row-block-major into
#     xh [128, 33*1024] so big column-range DMAs are per-partition
#     contiguous.  Band of W gathered into two stationary chunk tensors
#     (w1[:, O*128+j][i] = W[128O+j, 128O-32+i], w2 same at +96).
#   - Device (per core): x arrives in 7 large DMAs (0.5-1.5 MB) on the Sync
#     HWDGE ring; outputs leave in 8 x 1MB DMAs on the Scalar HWDGE ring;
#     weights via GpSimd SWDGE.  Per output tile O (32): 4 bf16 matmuls
#     (2 K-chunks x 2 batch halves) into 2 PSUM banks; evacuation split
#     between ScalarE (batch half 0) and VectorE (half 1), each doing
#     psum + bias -> bf16 out tile.
#   - Host: y = concat of per-core unpacked [1024, 4096] tiles, cast f32.
#
# kernel() is self-contained: shapes/sharding hardcoded, no file reads.

import ml_dtypes
import numpy as np

import concourse.mybir as mybir
import concourse.tile as tile
from concourse import bacc
from concourse.bass_utils import run_bass_kernel_spmd

BF16 = ml_dtypes.bfloat16

BATCH = 8192
IN = 4096
N_CORES = 8
B_CORE = BATCH // N_CORES          # 1024
O_TILES = IN // 128                # 32
PAD_TOP = 32
NBLK = O_TILES + 1                 # 33 x row-blocks of 128
BC = 512                           # batch chunk (one PSUM bank of f32)
N_BC = B_CORE // BC                # 2
OG = 4                             # output tiles per out-DMA group
# x DMA piece sizes in row-blocks (sum = NBLK); small first so compute starts
X_PIECES = [2, 3, 4, 6, 6, 6, 6]

_NC_CACHE = {}


def _build_nc():
    if "nc" in _NC_CACHE:
        return _NC_CACHE["nc"]
    f32 = mybir.dt.float32
    bf16 = mybir.dt.bfloat16
    nc = bacc.Bacc("TRN2", target_bir_lowering=False, debug=False)
    xh_d = nc.dram_tensor("xh", [128, NBLK * B_CORE], bf16, kind="ExternalInput")
    w1_d = nc.dram_tensor("w1", [128, IN], bf16, kind="ExternalInput")
    w2_d = nc.dram_tensor("w2", [128, IN], bf16, kind="ExternalInput")
    bias_d = nc.dram_tensor("bias", [128, O_TILES], f32, kind="ExternalInput")
    yt_d = nc.dram_tensor("yt", [128, O_TILES * B_CORE], bf16,
                          kind="ExternalOutput")

    with tile.TileContext(nc) as tc:
        with (
            tc.tile_pool(name="consts", bufs=1) as cpool,
            tc.tile_pool(name="psum", bufs=8, space="PSUM") as ppool,
            tc.tile_pool(name="out", bufs=8) as opool,
        ):
            # weights + bias on the Scalar HWDGE ring (parallel to x on Sync),
            # chunked so the first output tiles unblock early
            w1_t = cpool.tile([128, IN], bf16, name="w1", tag="w1")
            w2_t = cpool.tile([128, IN], bf16, name="w2", tag="w2")
            for q in range(2):
                qs = slice(q * (IN // 2), (q + 1) * (IN // 2))
                nc.scalar.dma_start(w1_t[:, qs], w1_d.ap()[:, qs])
                nc.scalar.dma_start(w2_t[:, qs], w2_d.ap()[:, qs])
            bias_t = cpool.tile([128, O_TILES], f32, name="bias", tag="bias")
            nc.scalar.dma_start(bias_t, bias_d.ap())

            # x pieces: large column-range DMAs on the Sync HWDGE ring
            xp_t = []          # per piece tile
            blk_loc = {}       # row-block -> (piece idx, local col offset)
            c0 = 0
            for pi, nb in enumerate(X_PIECES):
                t = cpool.tile([128, nb * B_CORE], bf16, name=f"xp{pi}",
                               tag=f"xp{pi}")
                nc.sync.dma_start(
                    t, xh_d.ap()[:, c0 * B_CORE:(c0 + nb) * B_CORE]
                )
                xp_t.append(t)
                for j in range(nb):
                    blk_loc[c0 + j] = (pi, j * B_CORE)
                c0 += nb

            def xblk(b, bc):
                pi, off = blk_loc[b]
                return xp_t[pi][:, off + bc * BC: off + (bc + 1) * BC]

            for g in range(O_TILES // OG):
                og_t = opool.tile([128, OG * B_CORE], bf16, tag="out")
                for lo in range(OG):
                    O = g * OG + lo
                    osl = slice(O * 128, (O + 1) * 128)
                    pss = [
                        ppool.tile([128, BC], f32, tag="ps", name=f"ps_{O}_{i}")
                        for i in range(N_BC)
                    ]
                    # one stationary load per chunk, both batch halves
                    for bc in range(N_BC):
                        nc.tensor.matmul(pss[bc], w1_t[:, osl], xblk(O, bc),
                                         start=True, stop=False)
                    for bc in range(N_BC):
                        nc.tensor.matmul(pss[bc], w2_t[:, osl], xblk(O + 1, bc),
                                         start=False, stop=True)
                    # evacuation: psum + bias -> bf16, split ACT / DVE
                    ob = lo * B_CORE
                    nc.scalar.add(og_t[:, ob:ob + BC], pss[0],
                                  bias_t[:, O:O + 1])
                    nc.vector.tensor_scalar_add(og_t[:, ob + BC:ob + 2 * BC],
                                                pss[1], bias_t[:, O:O + 1])
                # 1MB output DMA on the Sync ring: FIFO order behind the x
                # pieces defers output traffic until input streaming is done,
                # keeping the PE fed (and HAM-warm) during the compute phase
                nc.sync.dma_start(
                    yt_d.ap()[:, g * OG * B_CORE:(g + 1) * OG * B_CORE], og_t
                )

    nc.compile()
    _NC_CACHE["nc"] = nc
    return nc


def _band_gather(W, shift):
    """wc[i, O*128+j] = W[128O+j, 128O+shift+i], zero outside [0, IN)."""
    i = np.arange(128)[:, None, None]
    O = np.arange(O_TILES)[None, :, None]
    j = np.arange(128)[None, None, :]
    o_idx = np.broadcast_to(128 * O + j, (128, O_TILES, 128))
    f = 128 * O + shift + i
    wc = np.where(
        (f >= 0) & (f < IN), W[o_idx, np.clip(f, 0, IN - 1)], np.float32(0)
    )
    return wc.reshape(128, O_TILES * 128)


def kernel(x, W, b, mask=None):
    x = np.asarray(x, dtype=np.float32)
    W = np.asarray(W, dtype=np.float32)

    w1 = _band_gather(W, -PAD_TOP).astype(BF16)
    w2 = _band_gather(W, 128 - PAD_TOP).astype(BF16)
    bias = np.ascontiguousarray(
        np.asarray(b, dtype=np.float32).reshape(O_TILES, 128).T
    )

    xt = x.T  # [4096, 8192] view
    in_maps = []
    for c in range(N_CORES):
        sh = np.zeros((NBLK * 128, B_CORE), np.float32)
        sh[PAD_TOP:PAD_TOP + IN, :] = xt[:, c * B_CORE:(c + 1) * B_CORE]
        xh = np.ascontiguousarray(
            sh.astype(BF16).reshape(NBLK, 128, B_CORE).transpose(1, 0, 2)
        ).reshape(128, NBLK * B_CORE)
        in_maps.append({"xh": xh, "w1": w1, "w2": w2, "bias": bias})

    nc = _build_nc()
    res = run_bass_kernel_spmd(nc, in_maps, core_ids=list(range(N_CORES)))
    outs = []
    for r in res.results:
        yt = np.asarray(r["yt"]).reshape(128, O_TILES, B_CORE)
        outs.append(
            yt.transpose(2, 1, 0).reshape(B_CORE, IN).astype(np.float32)
        )
    return np.ascontiguousarray(np.concatenate(outs, axis=0))


if __name__ == "__main__":
    rng = np.random.default_rng(0)
    x = rng.standard_normal((BATCH, IN), dtype=np.float32)
    W = rng.standard_normal((IN, IN), dtype=np.float32)
    b = rng.standard_normal(IN, dtype=np.float32)
    y = kernel(x, W, b)
    print(y.shape, y.dtype)
